# revision 13
# baseline (speedup 1.0000x reference)
"""Trainium2 Bass kernel for nn_Custom_Pooling_3D.

Math (from the reference): the 0/1 matrix T encodes a fixed 2x2 spatial
sum-pool over a [I=32, J=32, C=16] layout (basis index i*512 + j*16 + c),
producing [O=16, O=16, C=16] (index oi*256 + oj*16 + c):

    y[b, oi, oj, c] = sqrt( sum_{di,dj in {0,1}} x[b, 2oi+di, 2oj+dj, c]^2 )

So T is never needed on device; the pooling structure is hardcoded.

Sharding: data-parallel over batch. 1024 rows / 8 cores = 128 rows per
core = exactly the 128 SBUF partitions. Per core: stream the 16384-wide
free dim in 8 chunks of 2048 (4 spatial i-rows), square on ScalarE
(ACT) + VectorE (DVE), two strided tensor_adds on DVE for the 2x2 sum,
sqrt on ACT, store. No cross-core communication.
"""

import os
import sys

import numpy as np

for _p in ("/opt/trn_rl_repo", "/root/.axon_site/_ro/trn_rl_repo"):
    if os.path.isdir(_p) and _p not in sys.path:
        sys.path.insert(0, _p)

import concourse.bass as bass
import concourse.tile as tile
from concourse import bacc, mybir
from concourse.bass_utils import run_bass_kernel_spmd

N_CORES = 8
BATCH = 1024
IN_F = 16384  # 32 * 32 * 16  (i, j, c)
OUT_F = 4096  # 16 * 16 * 16  (oi, oj, c)
BSH = BATCH // N_CORES  # 128 rows per core == SBUF partition count

# Input-column widths per chunk (each a multiple of 1024 so every chunk
# holds whole oi-pairs).  Front-loaded: big chunks stream while the pipe
# is DMA-bound; small chunks at the end shrink the serial drain tail.
CHUNKS = [4096, 4096, 3072, 3072, 1024, 1024]
BUFS = dict(xp=3, zp=2, tp=2, rp=2, op=3)

_CACHE = {}


def _build_program(chunks=None, bufs=None):
    chunks = chunks or CHUNKS
    bufs = bufs or BUFS
    assert sum(chunks) == IN_F and all(c % 1024 == 0 for c in chunks)

    # Bacc (not plain Bass): its compile() runs generate_event_semaphores,
    # which legalizes to TRN2's 1-wait-per-instruction limit.
    nc = bacc.Bacc("TRN2", target_bir_lowering=False, debug=False)
    f32 = mybir.dt.float32
    AF = mybir.ActivationFunctionType
    x = nc.dram_tensor("x", [BSH, IN_F], f32, kind="ExternalInput").ap()
    y = nc.dram_tensor("y", [BSH, OUT_F], f32, kind="ExternalOutput").ap()

    with tile.TileContext(nc) as tc:
        with (
            tc.tile_pool(name="xp", bufs=bufs["xp"]) as xp,
            tc.tile_pool(name="zp", bufs=bufs["zp"]) as zp,
            tc.tile_pool(name="tp", bufs=bufs["tp"]) as tp,
            tc.tile_pool(name="rp", bufs=bufs["rp"]) as rp,
            tc.tile_pool(name="op", bufs=bufs["op"]) as op,
        ):
            xoff = 0
            yoff = 0
            for ci, cin in enumerate(chunks):
                ni = cin // 512  # i-rows in this chunk
                cout = cin // 4
                xt = xp.tile([BSH, cin], f32, tag="xt")
                nc.sync.dma_start(xt[:, :], x[:, xoff : xoff + cin])

                # square on ACT (single writer per tile keeps sync waits low)
                zt = zp.tile([BSH, cin], f32, tag="zt")
                nc.scalar.activation(zt[:, :], xt[:, :], AF.Square)

                # j-pair add: [i, oj(16), 2, c(16)] -> [i, oj(16), c(16)]
                z = zt[:, :].rearrange(
                    "p (i oj two c) -> p i oj two c", i=ni, oj=16, two=2, c=16
                )
                tt = tp.tile([BSH, 2 * cout], f32, tag="tt")
                t4 = tt[:, :].rearrange("p (i oj c) -> p i oj c", i=ni, oj=16, c=16)
                nc.vector.tensor_add(t4, z[:, :, :, 0, :], z[:, :, :, 1, :])

                # i-pair add: [oi, 2, m(256)] -> [oi, m(256)]
                t3 = tt[:, :].rearrange(
                    "p (oi two m) -> p oi two m", oi=ni // 2, two=2, m=256
                )
                rt = rp.tile([BSH, cout], f32, tag="rt")
                r3 = rt[:, :].rearrange("p (oi m) -> p oi m", oi=ni // 2, m=256)
                nc.vector.tensor_add(r3, t3[:, :, 0, :], t3[:, :, 1, :])

                # sqrt to its own tile, then store
                ot = op.tile([BSH, cout], f32, tag="ot")
                nc.scalar.activation(ot[:, :], rt[:, :], AF.Sqrt)
                nc.sync.dma_start(y[:, yoff : yoff + cout], ot[:, :])
                xoff += cin
                yoff += cout
    nc.compile()
    _fuse_act_table_loads(nc, {AF.Square, AF.Sqrt})
    return nc


def _fuse_act_table_loads(nc, funcs_used):
    """bacc's insert_act_table_loads picks the first table set per function,
    which here yields two loads (square -> set 0, sqrt -> set 3) at ~2.7us
    each.  One set (sqrt_and_others) contains both; patch the first load to
    it and drop the rest.  Loads carry no sync info, so deletion is safe."""
    from concourse.hw_specs import get_activation_tables

    tabs = list(get_activation_tables(nc.m.arch).items())
    combined = next(
        (i for i, (_, fns) in enumerate(tabs) if funcs_used <= fns), None
    )
    if combined is None:
        return
    for blk in nc.m.functions[0].blocks:
        insts = blk.instructions  # live list view
        loads = [i for i in insts if type(i).__name__ == "InstLoadActFuncSet"]
        if len(loads) <= 1:
            continue
        if any(i.sync_info and (i.sync_info.on_wait or i.sync_info.on_update)
               for i in loads):
            continue
        loads[0].act_func_set_id = combined
        for extra in loads[1:]:
            insts.remove(extra)


def _run(x_full, trace=False, tmpdir=None):
    """x_full: [1024, 16384] f32. Returns (y_full [1024, 4096] f32, results obj)."""
    if "nc" not in _CACHE:
        _CACHE["nc"] = _build_program()
    nc = _CACHE["nc"]
    in_maps = [
        {"x": np.ascontiguousarray(x_full[c * BSH : (c + 1) * BSH])}
        for c in range(N_CORES)
    ]
    res = run_bass_kernel_spmd(
        nc, in_maps, list(range(N_CORES)), trace=trace, tmpdir=tmpdir
    )
    y_full = np.concatenate([res.results[c]["y"] for c in range(N_CORES)], axis=0)
    return y_full, res


def kernel(input_state, T=None, **_unused):
    x = np.asarray(input_state, dtype=np.float32)
    assert x.shape == (BATCH, IN_F), x.shape
    y, _ = _run(x, trace=False)
    return y


# revision 21
# speedup vs baseline: 1.0155x; 1.0155x over previous
"""Trainium2 Bass kernel for nn_Custom_Pooling_3D.

Math (from the reference): the 0/1 matrix T encodes a fixed 2x2 spatial
sum-pool over a [I=32, J=32, C=16] layout (basis index i*512 + j*16 + c),
producing [O=16, O=16, C=16] (index oi*256 + oj*16 + c):

    y[b, oi, oj, c] = sqrt( sum_{di,dj in {0,1}} x[b, 2oi+di, 2oj+dj, c]^2 )

So T is never needed on device; the pooling structure is hardcoded.

Sharding: data-parallel over batch. 1024 rows / 8 cores = 128 rows per
core = exactly the 128 SBUF partitions. Per core: stream the 16384-wide
free dim in tapered chunks (big first, small last to shrink the drain
tail), square on ScalarE (ACT), two strided tensor_adds on VectorE
(DVE) for the 2x2 window sum, sqrt on ACT, store. The kernel is
DMA-bound: ~10.5 MiB/core of HBM traffic at ~360 GB/s (~29 us floor);
ACT (~21 us) and DVE (~14 us) hide under it. No cross-core comms.
"""

import os
import sys

import numpy as np

for _p in ("/opt/trn_rl_repo", "/root/.axon_site/_ro/trn_rl_repo"):
    if os.path.isdir(_p) and _p not in sys.path:
        sys.path.insert(0, _p)

import concourse.tile as tile
from concourse import bacc, mybir
from concourse.bass_utils import run_bass_kernel_spmd

N_CORES = 8
BATCH = 1024
IN_F = 16384  # 32 * 32 * 16  (i, j, c)
OUT_F = 4096  # 16 * 16 * 16  (oi, oj, c)
BSH = BATCH // N_CORES  # 128 rows per core == SBUF partition count

# Input-column widths per chunk (each a multiple of 1024 so every chunk
# holds whole oi-pairs).  Front-loaded: big chunks stream while the pipe
# is DMA-bound; small chunks at the end shrink the serial drain tail.
CHUNKS = [4096, 4096, 4096, 3072, 1024]
BUFS = dict(xp=4, zp=3, tp=3, rp=3, op=4)

_CACHE = {}


def _build_program(chunks=None, bufs=None):
    chunks = chunks or CHUNKS
    bufs = bufs or BUFS
    assert sum(chunks) == IN_F and all(c % 1024 == 0 for c in chunks)

    # Bacc (not plain Bass): its compile() runs generate_event_semaphores,
    # which legalizes to TRN2's 1-wait-per-instruction limit.
    nc = bacc.Bacc("TRN2", target_bir_lowering=False, debug=False)
    f32 = mybir.dt.float32
    AF = mybir.ActivationFunctionType
    x = nc.dram_tensor("x", [BSH, IN_F], f32, kind="ExternalInput").ap()
    y = nc.dram_tensor("y", [BSH, OUT_F], f32, kind="ExternalOutput").ap()

    with tile.TileContext(nc) as tc:
        with (
            tc.tile_pool(name="xp", bufs=bufs["xp"]) as xp,
            tc.tile_pool(name="zp", bufs=bufs["zp"]) as zp,
            tc.tile_pool(name="tp", bufs=bufs["tp"]) as tp,
            tc.tile_pool(name="rp", bufs=bufs["rp"]) as rp,
            tc.tile_pool(name="op", bufs=bufs["op"]) as op,
        ):
            xoff = 0
            yoff = 0
            for cin in chunks:
                ni = cin // 512  # i-rows in this chunk
                cout = cin // 4
                xt = xp.tile([BSH, cin], f32, tag="xt")
                nc.sync.dma_start(xt[:, :], x[:, xoff : xoff + cin])

                # square on ACT (single writer per tile keeps sync waits low)
                zt = zp.tile([BSH, cin], f32, tag="zt")
                nc.scalar.activation(zt[:, :], xt[:, :], AF.Square)

                # j-pair add: [i, oj(16), 2, c(16)] -> [i, oj(16), c(16)]
                z = zt[:, :].rearrange(
                    "p (i oj two c) -> p i oj two c", i=ni, oj=16, two=2, c=16
                )
                tt = tp.tile([BSH, 2 * cout], f32, tag="tt")
                t4 = tt[:, :].rearrange("p (i oj c) -> p i oj c", i=ni, oj=16, c=16)
                nc.vector.tensor_add(t4, z[:, :, :, 0, :], z[:, :, :, 1, :])

                # i-pair add: [oi, 2, m(256)] -> [oi, m(256)]
                t3 = tt[:, :].rearrange(
                    "p (oi two m) -> p oi two m", oi=ni // 2, two=2, m=256
                )
                rt = rp.tile([BSH, cout], f32, tag="rt")
                r3 = rt[:, :].rearrange("p (oi m) -> p oi m", oi=ni // 2, m=256)
                nc.vector.tensor_add(r3, t3[:, :, 0, :], t3[:, :, 1, :])

                # sqrt to its own tile, then store
                ot = op.tile([BSH, cout], f32, tag="ot")
                nc.scalar.activation(ot[:, :], rt[:, :], AF.Sqrt)
                nc.sync.dma_start(y[:, yoff : yoff + cout], ot[:, :])
                xoff += cin
                yoff += cout
    nc.compile()
    _fuse_act_table_loads(nc, {AF.Square, AF.Sqrt})
    return nc


def _fuse_act_table_loads(nc, funcs_used):
    """bacc's insert_act_table_loads picks the first table set per function,
    which here yields two loads (square -> set 0, sqrt -> set 3) at ~2.7us
    each.  One set (sqrt_and_others) contains both; patch the first load to
    it and drop the rest.  Loads carry no sync info, so deletion is safe."""
    from concourse.hw_specs import get_activation_tables

    tabs = list(get_activation_tables(nc.m.arch).items())
    combined = next(
        (i for i, (_, fns) in enumerate(tabs) if funcs_used <= fns), None
    )
    if combined is None:
        return
    for blk in nc.m.functions[0].blocks:
        insts = blk.instructions  # live list view
        loads = [i for i in insts if type(i).__name__ == "InstLoadActFuncSet"]
        if len(loads) <= 1:
            continue
        if any(i.sync_info and (i.sync_info.on_wait or i.sync_info.on_update)
               for i in loads):
            continue
        loads[0].act_func_set_id = combined
        for extra in loads[1:]:
            insts.remove(extra)


def _run(x_full, trace=False, tmpdir=None):
    """x_full: [1024, 16384] f32. Returns (y_full [1024, 4096] f32, results obj)."""
    if "nc" not in _CACHE:
        _CACHE["nc"] = _build_program()
    nc = _CACHE["nc"]
    in_maps = [
        {"x": np.ascontiguousarray(x_full[c * BSH : (c + 1) * BSH])}
        for c in range(N_CORES)
    ]
    res = run_bass_kernel_spmd(
        nc, in_maps, list(range(N_CORES)), trace=trace, tmpdir=tmpdir
    )
    y_full = np.concatenate([res.results[c]["y"] for c in range(N_CORES)], axis=0)
    return y_full, res


def kernel(input_state, T=None, **_unused):
    x = np.asarray(input_state, dtype=np.float32)
    assert x.shape == (BATCH, IN_F), x.shape
    y, _ = _run(x, trace=False)
    return y


# revision 35
# speedup vs baseline: 1.0712x; 1.0548x over previous
"""Trainium2 Bass kernel for nn_Custom_Pooling_3D.

Math (from the reference): the 0/1 matrix T encodes a fixed 2x2 spatial
sum-pool over a [I=32, J=32, C=16] layout (basis index i*512 + j*16 + c),
producing [O=16, O=16, C=16] (index oi*256 + oj*16 + c):

    y[b, oi, oj, c] = sqrt( sum_{di,dj in {0,1}} x[b, 2oi+di, 2oj+dj, c]^2 )

So T is never needed on device; the pooling structure is hardcoded.

Sharding: data-parallel over batch. 1024 rows / 8 cores = 128 rows per
core = exactly the 128 SBUF partitions. Per core: stream the 16384-wide
free dim in tapered chunks (big first, small last to shrink the drain
tail), square on ScalarE (ACT), two strided tensor_adds on VectorE
(DVE) for the 2x2 window sum, sqrt on ACT, store. The kernel is
DMA-bound: ~10.5 MiB/core of HBM traffic at ~360 GB/s (~29 us floor);
ACT (~21 us) and DVE (~14 us) hide under it. No cross-core comms.

Overlap notes: engine sequencers dispatch serially and block inside a
wait, so loads (whose waits clear early) and stores (which wait on
compute) must not share a sequencer — early stores go to the Pool/SWDGE
sequencer, the last few back on the by-then-drained SP. Load-pool depth
xp=5 makes tail loads' slot-reuse waits fire before store triggers so
the final load is not queued behind store transfers.
"""

import os
import sys

import numpy as np

for _p in ("/opt/trn_rl_repo", "/root/.axon_site/_ro/trn_rl_repo"):
    if os.path.isdir(_p) and _p not in sys.path:
        sys.path.insert(0, _p)

import concourse.tile as tile
from concourse import bacc, mybir
from concourse.bass_utils import run_bass_kernel_spmd

N_CORES = 8
BATCH = 1024
IN_F = 16384  # 32 * 32 * 16  (i, j, c)
OUT_F = 4096  # 16 * 16 * 16  (oi, oj, c)
BSH = BATCH // N_CORES  # 128 rows per core == SBUF partition count

# Input-column widths per chunk (each a multiple of 1024 so every chunk
# holds whole oi-pairs).  Front-loaded: big chunks stream while the pipe
# is DMA-bound; small chunks at the end shrink the serial drain tail.
CHUNKS = [4096, 3072, 3072, 2048, 2048, 1024, 1024]
# xp=5 matters: with fewer slots the tail loads' slot-WAR waits fire late
# and their DMA triggers lose the engine to earlier-triggered stores.
BUFS = dict(xp=5, zp=2, tp=3, rp=3, op=4)
# Early stores dispatch from the idle Pool sequencer (SWDGE) so their
# sqrt-waits can't head-of-line-block load dispatches on SP; the last few
# go back on SP, which is drained by then, to get HWDGE latency for the
# critical final stores.
STORE_ENGS = ["gpsimd"] * 4 + ["sync"] * 3

_CACHE = {}


def _build_program(chunks=None, bufs=None, store_engs=None, sq_engs=None):
    chunks = chunks or CHUNKS
    bufs = bufs or BUFS
    assert sum(chunks) == IN_F and all(c % 1024 == 0 for c in chunks)
    if sq_engs is None:
        sq_engs = ["scalar"] * len(chunks)
    if store_engs is None:
        store_engs = STORE_ENGS if chunks is CHUNKS or chunks == CHUNKS else (
            ["gpsimd"] * (len(chunks) - 1) + ["sync"]
        )

    # Bacc (not plain Bass): its compile() runs generate_event_semaphores,
    # which legalizes to TRN2's 1-wait-per-instruction limit.
    nc = bacc.Bacc("TRN2", target_bir_lowering=False, debug=False)
    f32 = mybir.dt.float32
    AF = mybir.ActivationFunctionType
    x = nc.dram_tensor("x", [BSH, IN_F], f32, kind="ExternalInput").ap()
    y = nc.dram_tensor("y", [BSH, OUT_F], f32, kind="ExternalOutput").ap()

    with tile.TileContext(nc) as tc:
        with (
            tc.tile_pool(name="xp", bufs=bufs["xp"]) as xp,
            tc.tile_pool(name="zp", bufs=bufs["zp"]) as zp,
            tc.tile_pool(name="tp", bufs=bufs["tp"]) as tp,
            tc.tile_pool(name="rp", bufs=bufs["rp"]) as rp,
            tc.tile_pool(name="op", bufs=bufs["op"]) as op,
        ):
            xoff = 0
            yoff = 0
            for idx, (cin, store_eng, sq_eng) in enumerate(
                zip(chunks, store_engs, sq_engs)
            ):
                ni = cin // 512  # i-rows in this chunk
                cout = cin // 4
                xt = xp.tile([BSH, cin], f32, tag="xt")
                nc.sync.dma_start(xt[:, :], x[:, xoff : xoff + cin])

                # square (single writer per tile keeps sync waits low)
                zt = zp.tile([BSH, cin], f32, tag="zt")
                if sq_eng == "vector":
                    nc.vector.tensor_mul(zt[:, :], xt[:, :], xt[:, :])
                else:
                    nc.scalar.activation(zt[:, :], xt[:, :], AF.Square)

                # j-pair add: [i, oj(16), 2, c(16)] -> [i, oj(16), c(16)]
                z = zt[:, :].rearrange(
                    "p (i oj two c) -> p i oj two c", i=ni, oj=16, two=2, c=16
                )
                tt = tp.tile([BSH, 2 * cout], f32, tag="tt")
                t4 = tt[:, :].rearrange("p (i oj c) -> p i oj c", i=ni, oj=16, c=16)
                nc.vector.tensor_add(t4, z[:, :, :, 0, :], z[:, :, :, 1, :])

                # i-pair add: [oi, 2, m(256)] -> [oi, m(256)]
                t3 = tt[:, :].rearrange(
                    "p (oi two m) -> p oi two m", oi=ni // 2, two=2, m=256
                )
                rt = rp.tile([BSH, cout], f32, tag="rt")
                r3 = rt[:, :].rearrange("p (oi m) -> p oi m", oi=ni // 2, m=256)
                nc.vector.tensor_add(r3, t3[:, :, 0, :], t3[:, :, 1, :])

                # sqrt to its own tile, then store (engine per store_engs)
                ot = op.tile([BSH, cout], f32, tag="ot")
                nc.scalar.activation(ot[:, :], rt[:, :], AF.Sqrt)
                getattr(nc, store_eng).dma_start(
                    y[:, yoff : yoff + cout], ot[:, :]
                )
                xoff += cin
                yoff += cout
    nc.compile()
    _fuse_act_table_loads(nc, {AF.Square, AF.Sqrt})
    return nc


def _fuse_act_table_loads(nc, funcs_used):
    """bacc's insert_act_table_loads picks the first table set per function,
    which here yields two loads (square -> set 0, sqrt -> set 3) at ~2.7us
    each.  One set (sqrt_and_others) contains both; patch the first load to
    it and drop the rest.  Loads carry no sync info, so deletion is safe."""
    from concourse.hw_specs import get_activation_tables

    tabs = list(get_activation_tables(nc.m.arch).items())
    combined = next(
        (i for i, (_, fns) in enumerate(tabs) if funcs_used <= fns), None
    )
    if combined is None:
        return
    for blk in nc.m.functions[0].blocks:
        insts = blk.instructions  # live list view
        loads = [i for i in insts if type(i).__name__ == "InstLoadActFuncSet"]
        if len(loads) <= 1:
            continue
        if any(i.sync_info and (i.sync_info.on_wait or i.sync_info.on_update)
               for i in loads):
            continue
        loads[0].act_func_set_id = combined
        for extra in loads[1:]:
            insts.remove(extra)


def _run(x_full, trace=False, tmpdir=None):
    """x_full: [1024, 16384] f32. Returns (y_full [1024, 4096] f32, results obj)."""
    if "nc" not in _CACHE:
        _CACHE["nc"] = _build_program()
    nc = _CACHE["nc"]
    in_maps = [
        {"x": np.ascontiguousarray(x_full[c * BSH : (c + 1) * BSH])}
        for c in range(N_CORES)
    ]
    res = run_bass_kernel_spmd(
        nc, in_maps, list(range(N_CORES)), trace=trace, tmpdir=tmpdir
    )
    y_full = np.concatenate([res.results[c]["y"] for c in range(N_CORES)], axis=0)
    return y_full, res


def kernel(input_state, T=None, **_unused):
    x = np.asarray(input_state, dtype=np.float32)
    assert x.shape == (BATCH, IN_F), x.shape
    y, _ = _run(x, trace=False)
    return y
